# revision 12
# baseline (speedup 1.0000x reference)
"""MoE transformer block on 8 TRN2 NeuronCores (self-contained).

Sharding: tokens split 8 ways as (batch, seq-half) -> 512 tokens/core for
attention (data-parallel, fp32 matmuls so the top-2 routing decisions match
the f32 reference bit-for-bit-ish); experts split 1/core (expert-parallel,
bf16 FFN).  Cross-core collectives: AllGather-8 of K/V, of ln2 activations
and of dense gate weights; ReduceScatter(add) of expert outputs.

On-device token routing: top-2 via Max8; compaction via free-dim prefix scan
+ triangular-matmul partition prefix; (token-id, gate) pairs scattered into
an HBM table by slot; token rows gathered / expert outputs scattered back by
indirect DMA.
"""
from contextlib import ExitStack

import os
import numpy as np
import ml_dtypes
import concourse.bass as bass
import concourse.bacc as bacc
import concourse.mybir as mybir
import concourse.tile as tile
from concourse.bass_utils import run_bass_kernel_spmd
from concourse.masks import make_identity

P = 128
NC = 8
D = 1024
H = 16
HD = 64
F = 4096
E = 8
B = 4
S = 1024
TOK = 512              # tokens owned per core
NTOK = 4096
TT = TOK // P          # 4 token tiles per core
DC = D // P            # 8 contraction chunks of 128
FJ = F // P            # 32 ffn-dim tiles
CAP = 1280             # expert slot capacity (dump slot = CAP)
NG = CAP // P          # 10 slot groups of 128
SCH = 2                # slot groups per FFN chunk (256 slots)
J = NTOK // P          # 32 tokens per partition in routing layout
NEG = -1e30
EPS = 1e-5

f32 = mybir.dt.float32
bf16 = mybir.dt.bfloat16
i32 = mybir.dt.int32
AF = mybir.ActivationFunctionType
ALU = mybir.AluOpType
AX = mybir.AxisListType
RG8 = [list(range(NC))]


def build():
    nc = bacc.Bacc()
    dp = nc.declare_dram_parameter
    # per-core inputs
    xown = dp("xown", [TOK, D], f32, isOutput=False)
    maskt = dp("maskt", [S, TOK], bf16, isOutput=False)       # additive [k, q]
    kvsel = dp("kvsel", [P, DC], i32, isOutput=False)         # kv_full row ids
    sel1 = dp("sel1", [P, E], f32, isOutput=False)            # expert onehot (replicated)
    # replicated params
    ln1w = dp("ln1w", [P, D], f32, isOutput=False)
    ln1b = dp("ln1b", [P, D], f32, isOutput=False)
    ln2w = dp("ln2w", [P, D], f32, isOutput=False)
    ln2b = dp("ln2b", [P, D], f32, isOutput=False)
    wqkv = dp("wqkv", [D, 3 * D], f32, isOutput=False)
    bq_pj = dp("bq_pj", [P, DC], f32, isOutput=False)         # Q bias, feat=128*j+p
    bkv = dp("bkv", [P, 2 * D], f32, isOutput=False)          # K,V bias (replicated row)
    wo = dp("wo", [D, D], f32, isOutput=False)
    bo = dp("bo", [P, D], f32, isOutput=False)
    gatew = dp("gatew", [D, E], f32, isOutput=False)
    ltri = dp("ltri", [P, P], f32, isOutput=False)            # LT[p',p]=1 iff p'<p
    # per-core expert weights
    fc1wt = dp("fc1wt", [D, F], bf16, isOutput=False)
    fc1b_pj = dp("fc1b_pj", [P, FJ], f32, isOutput=False)     # f = 128*j+p
    fc2wt = dp("fc2wt", [F, D], bf16, isOutput=False)
    fc2b = dp("fc2b", [P, D], f32, isOutput=False)
    out = dp("out", [TOK, D], f32, isOutput=True)
    dbg = os.environ.get("KERNEL_DEBUG_TAPS") == "1"
    if dbg:
        d_x2 = dp("d_x2", [TOK, D], f32, isOutput=True)
        d_nx2 = dp("d_nx2", [NTOK, D], bf16, isOutput=True)
        d_g = dp("d_g", [NTOK, E], f32, isOutput=True)
        d_tbl = dp("d_tbl", [CAP + 1, 2], f32, isOutput=True)
        d_rs = dp("d_rs", [TOK, D], bf16, isOutput=True)
        d_kv = dp("d_kv", [TOK, 2 * D], f32, isOutput=True)

    # internal DRAM
    kv_send = nc.dram_tensor("kv_send", [TOK, 2 * D], f32)
    kv_full = nc.dram_tensor("kv_full", [NTOK, 2 * D], f32, addr_space="Shared")
    nx2_send = nc.dram_tensor("nx2_send", [TOK, D], bf16)
    nx2_full = nc.dram_tensor("nx2_full", [NTOK, D], bf16, addr_space="Shared")
    g_send = nc.dram_tensor("g_send", [TOK, E], f32)
    g_full = nc.dram_tensor("g_full", [NTOK, E], f32, addr_space="Shared")
    table = nc.dram_tensor("table", [CAP + 1, 2], f32)
    y_full = nc.dram_tensor("y_full", [NTOK + 1, D], bf16)
    rs_out = nc.dram_tensor("rs_out", [TOK, D], bf16)
    x2_dram = nc.dram_tensor("x2_dram", [TOK, D], f32)
    qt_dram = nc.dram_tensor("qt_dram", [D, TOK], f32)

    with tile.TileContext(nc) as tc, ExitStack() as top:
        cst = top.enter_context(tc.tile_pool(name="cst", bufs=1))

        # ---- constants / init
        ident = cst.tile([P, P], f32)
        make_identity(nc, ident[:, :])
        identb = cst.tile([P, P], bf16)
        make_identity(nc, identb[:, :])
        tinit = cst.tile([P, NG + 1, 2], f32)
        nc.vector.memset(tinit[:, :, 0:1], float(NTOK))
        nc.vector.memset(tinit[:, :, 1:2], 0.0)
        nc.sync.dma_start(out=table[:CAP].rearrange("(g p) c -> p g c", p=P),
                          in_=tinit[:, :NG, :])
        nc.sync.dma_start(out=table[CAP:CAP + 1, :], in_=tinit[0:1, NG, :])
        zrow = cst.tile([P, D], bf16)
        nc.vector.memset(zrow[:], 0.0)
        for k in range(NTOK // P):
            nc.sync.dma_start(out=y_full[k * P:(k + 1) * P, :], in_=zrow[:, :])
        lt_sb = cst.tile([P, P], f32)
        nc.sync.dma_start(out=lt_sb[:], in_=ltri[:, :])
        sel1_sb = cst.tile([P, E], f32)
        nc.sync.dma_start(out=sel1_sb[:], in_=sel1[:, :])
        ids_i = cst.tile([P, NG], i32)
        gslot = cst.tile([P, NG], f32)

        def layernorm(src, dst, wrow_d, brow_d, tag, pool, lns):
            wrow = pool.tile([P, D], f32, tag=tag + "w")
            brow = pool.tile([P, D], f32, tag=tag + "b")
            nc.sync.dma_start(out=wrow[:], in_=wrow_d[:, :])
            nc.sync.dma_start(out=brow[:], in_=brow_d[:, :])
            for t in range(TT):
                mu = lns.tile([P, 1], f32, tag="ln_mu")
                nc.vector.tensor_reduce(mu[:], src[:, t, :], axis=AX.X, op=ALU.add)
                nc.vector.tensor_scalar_mul(mu[:], mu[:], 1.0 / D)
                xc = lns.tile([P, D], f32, tag="ln_xc")
                nc.vector.tensor_scalar_sub(xc[:], src[:, t, :], mu[:, 0:1])
                sq = lns.tile([P, D], f32, tag="ln_sq")
                ssq = lns.tile([P, 1], f32, tag="ln_ssq")
                nc.scalar.activation(sq[:], xc[:], AF.Square, accum_out=ssq[:])
                nc.vector.tensor_scalar(ssq[:], ssq[:], 1.0 / D, EPS, ALU.mult, ALU.add)
                nc.scalar.sqrt(ssq[:], ssq[:])
                rstd = lns.tile([P, 1], f32, tag="ln_rstd")
                nc.vector.reciprocal(rstd[:], ssq[:])
                nc.vector.tensor_scalar_mul(xc[:], xc[:], rstd[:, 0:1])
                nc.vector.tensor_mul(xc[:], xc[:], wrow[:, :])
                nc.vector.tensor_add(dst[:, t, :], xc[:], brow[:, :])

        # ======== Phase A: LN1 + QKV (fp32) ========
        with ExitStack() as ph:
            pA = ph.enter_context(tc.tile_pool(name="pA", bufs=1))
            lnsA = ph.enter_context(tc.tile_pool(name="lnsA", bufs=2))
            wqp = ph.enter_context(tc.tile_pool(name="wqp", bufs=3))
            psA = ph.enter_context(tc.tile_pool(name="psA", bufs=2, space="PSUM"))
            psQ = ph.enter_context(tc.tile_pool(name="psQ", bufs=3, space="PSUM"))

            X = pA.tile([P, TT, D], f32)
            nc.sync.dma_start(out=X[:, :, :],
                              in_=xown.rearrange("(t p) d -> p t d", p=P))
            nx = pA.tile([P, TT, D], f32)
            layernorm(X, nx, ln1w, ln1b, "ln1", pA, lnsA)
            nxT = pA.tile([P, DC, TOK], f32)
            for dc in range(DC):
                for t in range(TT):
                    tp = psA.tile([P, P], f32, tag="tpose", space="PSUM")
                    nc.tensor.transpose(tp[:], nx[:, t, dc * P:(dc + 1) * P], ident[:, :])
                    nc.scalar.activation(nxT[:, dc, t * P:(t + 1) * P], tp[:], AF.Copy)

            bq_sb = pA.tile([P, DC], f32)
            nc.sync.dma_start(out=bq_sb[:], in_=bq_pj[:, :])
            for fc in range(DC):
                wq_sb = wqp.tile([P, DC, P], f32, tag="wq")
                nc.sync.dma_start(
                    out=wq_sb[:, :, :],
                    in_=wqkv[:, fc * P:(fc + 1) * P].rearrange("(c p) f -> p c f", p=P))
                ps = psQ.tile([P, TOK], f32, tag="qkv", space="PSUM")
                for dc in range(DC):
                    nc.tensor.matmul(ps[:], wq_sb[:, dc, :], nxT[:, dc, :],
                                     start=(dc == 0), stop=(dc == DC - 1))
                qtev = wqp.tile([P, TOK], f32, tag="qtev")
                nc.vector.tensor_scalar(qtev[:], ps[:], bq_sb[:, fc:fc + 1],
                                        1.0 / np.sqrt(HD), ALU.add, ALU.mult)
                nc.sync.dma_start(out=qt_dram[fc * P:(fc + 1) * P, :], in_=qtev[:])

            bkv_sb = pA.tile([P, 2 * D], f32)
            nc.sync.dma_start(out=bkv_sb[:], in_=bkv[:, :])
            kv_send_r = kv_send.rearrange("(t p) f -> p t f", p=P)
            for c2 in range(4):
                wkv_sb = wqp.tile([P, DC, TOK], f32, tag="wkv")
                nc.sync.dma_start(
                    out=wkv_sb[:, :, :],
                    in_=wqkv[:, D + c2 * TOK:D + (c2 + 1) * TOK]
                        .rearrange("(c p) f -> p c f", p=P))
                for t in range(TT):
                    ps = psQ.tile([P, TOK], f32, tag="qkv", space="PSUM")
                    for dc in range(DC):
                        nc.tensor.matmul(ps[:], nxT[:, dc, t * P:(t + 1) * P],
                                         wkv_sb[:, dc, :],
                                         start=(dc == 0), stop=(dc == DC - 1))
                    kvev = wqp.tile([P, TOK], f32, tag="kvev")
                    nc.vector.tensor_add(kvev[:], ps[:],
                                         bkv_sb[:, c2 * TOK:(c2 + 1) * TOK])
                    nc.sync.dma_start(out=kv_send_r[:, t, c2 * TOK:(c2 + 1) * TOK],
                                      in_=kvev[:])
        nc.gpsimd.collective_compute("AllGather", ALU.bypass, replica_groups=RG8,
                                     ins=[kv_send[:, :]], outs=[kv_full[:, :]])

        # ======== Phase B: attention (fp32, transposed-score form) ========
        px2 = tc.tile_pool(name="px2", bufs=1)
        px2pool = px2.__enter__()
        with ExitStack() as ph:
            psB = ph.enter_context(tc.tile_pool(name="psB", bufs=2, space="PSUM"))
            pAO = ph.enter_context(tc.tile_pool(name="pAO", bufs=1))
            AOT = pAO.tile([P, DC, TOK], f32)
            hs = ExitStack()
            psST = hs.enter_context(tc.tile_pool(name="psST", bufs=3, space="PSUM"))
            psAV = hs.enter_context(tc.tile_pool(name="psAV", bufs=2, space="PSUM"))

            kvsel_sb = cst.tile([P, DC], i32)
            nc.sync.dma_start(out=kvsel_sb[:], in_=kvsel[:, :])
            pKT = hs.enter_context(tc.tile_pool(name="pKT", bufs=1))
            KT = pKT.tile([P, DC, S], f32)
            Vext = pKT.tile([P, DC, H, HD + 1], f32)
            nc.vector.memset(Vext[:, :, :, HD:HD + 1], 1.0)
            with ExitStack() as phk:
                pKV = phk.enter_context(tc.tile_pool(name="pKV", bufs=1))
                for hb in range(2):
                    kvh = pKV.tile([P, DC // 2, 2 * D], f32, tag="kvh")
                    for tl in range(DC // 2):
                        t = hb * (DC // 2) + tl
                        nc.gpsimd.indirect_dma_start(
                            out=kvh[:, tl, :], out_offset=None,
                            in_=kv_full[:, :],
                            in_offset=bass.IndirectOffsetOnAxis(
                                ap=kvsel_sb[:, t:t + 1], axis=0))
                    for tl in range(DC // 2):
                        t = hb * (DC // 2) + tl
                        for dc in range(DC):
                            tp = psB.tile([P, P], f32, tag="tposeB", space="PSUM")
                            nc.tensor.transpose(tp[:], kvh[:, tl, dc * P:(dc + 1) * P],
                                                ident[:, :])
                            nc.scalar.activation(KT[:, dc, t * P:(t + 1) * P], tp[:],
                                                 AF.Copy)
                        for h in range(H):
                            nc.scalar.activation(Vext[:, t, h, :HD],
                                                 kvh[:, tl, D + h * HD:D + (h + 1) * HD],
                                                 AF.Copy)

            maskt_sb = pKT.tile([P, DC, TOK], bf16)
            nc.sync.dma_start(out=maskt_sb[:, :, :],
                              in_=maskt.rearrange("(t p) q -> p t q", p=P))

            etp = hs.enter_context(tc.tile_pool(name="etp", bufs=3))
            qtp = hs.enter_context(tc.tile_pool(name="qtp", bufs=2))
            for h in range(H):
                po = (h % 2) * HD
                ft = h // 2
                if po == 0:
                    qt_sb = qtp.tile([P, TOK], f32, tag="qt")
                    nc.sync.dma_start(out=qt_sb[:, :],
                                      in_=qt_dram[ft * P:(ft + 1) * P, :])
                av = psAV.tile([P, TOK], f32, tag="av", space="PSUM")
                for kt in range(DC):
                    st = psST.tile([P, TOK], f32, tag="st", space="PSUM")
                    nc.tensor.matmul(st[:], KT[po:po + HD, ft, kt * P:(kt + 1) * P],
                                     qt_sb[po:po + HD, :], start=True, stop=True)
                    sm = etp.tile([P, TOK], f32, tag="sm")
                    nc.vector.tensor_add(sm[:], st[:], maskt_sb[:, kt, :])
                    et = etp.tile([P, TOK], f32, tag="et")
                    nc.scalar.activation(et[:], sm[:], AF.Exp)
                    nc.tensor.matmul(av[:HD + 1, :], Vext[:, kt, h, :], et[:],
                                     start=(kt == 0), stop=(kt == DC - 1))
                rec = etp.tile([1, TOK], f32, tag="rec")
                nc.vector.reciprocal(rec[:], av[HD:HD + 1, :])
                recb = etp.tile([HD, TOK], f32, tag="recb")
                nc.gpsimd.partition_broadcast(recb[:, :], rec[0:1, :], channels=HD)
                nc.vector.tensor_mul(AOT[po:po + HD, ft, :], av[:HD, :], recb[:, :])
            hs.close()
            psP = ph.enter_context(tc.tile_pool(name="psP", bufs=2, space="PSUM"))

            # proj + residual -> x2
            wo_sb = pAO.tile([P, DC, D], f32)
            nc.sync.dma_start(out=wo_sb[:, :, :],
                              in_=wo.rearrange("(c p) f -> p c f", p=P))
            bo_sb = pAO.tile([P, D], f32)
            nc.sync.dma_start(out=bo_sb[:], in_=bo[:, :])
            x2 = px2pool.tile([P, TT, D], f32)
            xrp = ph.enter_context(tc.tile_pool(name="xrp", bufs=2))
            for t in range(TT):
                xr = xrp.tile([P, D], f32, tag="xr")
                nc.sync.dma_start(
                    out=xr[:, :],
                    in_=xown.rearrange("(t p) d -> p t d", p=P)[:, t, :])
                for fc in range(2):
                    ps = psP.tile([P, TOK], f32, tag="proj", space="PSUM")
                    for dc in range(DC):
                        nc.tensor.matmul(ps[:], AOT[:, dc, t * P:(t + 1) * P],
                                         wo_sb[:, dc, fc * TOK:(fc + 1) * TOK],
                                         start=(dc == 0), stop=(dc == DC - 1))
                    sl = slice(fc * TOK, (fc + 1) * TOK)
                    nc.vector.tensor_add(x2[:, t, sl], ps[:], bo_sb[:, sl])
                    nc.vector.tensor_add(x2[:, t, sl], x2[:, t, sl], xr[:, sl])
        nc.sync.dma_start(out=x2_dram.rearrange("(t p) d -> p t d", p=P), in_=x2[:, :, :])

        # ======== Phase C: LN2, gate, AGs ========
        with ExitStack() as ph:
            pC = ph.enter_context(tc.tile_pool(name="pC", bufs=1))
            lnsC = ph.enter_context(tc.tile_pool(name="lnsC", bufs=2))
            psC = ph.enter_context(tc.tile_pool(name="psC", bufs=2, space="PSUM"))
            gsc = ph.enter_context(tc.tile_pool(name="gsc", bufs=2))

            nx2 = pC.tile([P, TT, D], f32)
            layernorm(x2, nx2, ln2w, ln2b, "ln2", pC, lnsC)
            nx2b = pC.tile([P, TT, D], bf16)
            nc.vector.tensor_copy(nx2b[:, :, :], nx2[:, :, :])
            nc.sync.dma_start(out=nx2_send.rearrange("(t p) d -> p t d", p=P),
                              in_=nx2b[:, :, :])
            nc.gpsimd.collective_compute("AllGather", ALU.bypass, replica_groups=RG8,
                                         ins=[nx2_send[:, :]], outs=[nx2_full[:, :]])

            nx2T = pC.tile([P, DC, TOK], f32)
            for dc in range(DC):
                for t in range(TT):
                    tp = psC.tile([P, P], f32, tag="tposeC", space="PSUM")
                    nc.tensor.transpose(tp[:], nx2[:, t, dc * P:(dc + 1) * P], ident[:, :])
                    nc.scalar.activation(nx2T[:, dc, t * P:(t + 1) * P], tp[:], AF.Copy)
            gw_sb = pC.tile([P, DC, E], f32)
            nc.sync.dma_start(out=gw_sb[:, :, :],
                              in_=gatew.rearrange("(c p) e -> p c e", p=P))
            gden = pC.tile([P, TT, E], f32)
            for t in range(TT):
                ps = psC.tile([P, E], f32, tag="gate", space="PSUM")
                for dc in range(DC):
                    nc.tensor.matmul(ps[:], nx2T[:, dc, t * P:(t + 1) * P],
                                     gw_sb[:, dc, :],
                                     start=(dc == 0), stop=(dc == DC - 1))
                glog = gsc.tile([P, E], f32, tag="glog")
                nc.vector.tensor_copy(glog[:], ps[:])
                mx = gsc.tile([P, 8], f32, tag="mx")
                nc.vector.max(mx[:, :], glog[:, :])
                dlt = gsc.tile([P, E], f32, tag="dlt")
                nc.vector.tensor_scalar_sub(dlt[:], glog[:], mx[:, 0:1])
                ex = gsc.tile([P, E], f32, tag="ex")
                nc.scalar.activation(ex[:], dlt[:], AF.Exp)
                em2 = gsc.tile([P, 1], f32, tag="em2")
                nc.vector.tensor_sub(em2[:], mx[:, 1:2], mx[:, 0:1])
                nc.scalar.activation(em2[:], em2[:], AF.Exp)
                nc.vector.tensor_scalar_add(em2[:], em2[:], 1.0)
                rec2 = gsc.tile([P, 1], f32, tag="rec2")
                nc.vector.reciprocal(rec2[:], em2[:])
                nc.vector.tensor_scalar_mul(ex[:], ex[:], rec2[:, 0:1])
                msk = gsc.tile([P, E], f32, tag="msk")
                nc.vector.tensor_scalar(msk[:], glog[:], mx[:, 1:2], None, ALU.is_ge)
                nc.vector.tensor_mul(gden[:, t, :], ex[:], msk[:])
            nc.sync.dma_start(out=g_send.rearrange("(t p) e -> p t e", p=P),
                              in_=gden[:, :, :])
            nc.gpsimd.collective_compute("AllGather", ALU.bypass, replica_groups=RG8,
                                         ins=[g_send[:, :]], outs=[g_full[:, :]])

            # ---- routing (expert = this core); fills ids_i / gslot (cst pool)
            rt = ph.enter_context(tc.tile_pool(name="rt", bufs=1))
            gfull_sb = rt.tile([P, J, E], f32)
            nc.sync.dma_start(out=gfull_sb[:, :, :],
                              in_=g_full.rearrange("(p j) e -> p j e", p=P))
            gsel = rt.tile([P, J, E], f32)
            nc.vector.tensor_mul(gsel[:, :, :], gfull_sb[:, :, :],
                                 sel1_sb[:, :].unsqueeze(1).to_broadcast([P, J, E]))
            ge = rt.tile([P, J], f32)
            nc.vector.tensor_reduce(ge[:, :], gsel[:, :, :], axis=AX.X, op=ALU.add)
            selm = rt.tile([P, J], f32)
            nc.vector.tensor_scalar(selm[:], ge[:], 0.0, None, ALU.is_gt)
            csum = rt.tile([P, J], f32)
            nc.vector.tensor_tensor_scan(csum[:], selm[:], selm[:], 0.0,
                                         ALU.add, ALU.bypass)
            ppf_ps = psC.tile([P, 1], f32, tag="gate", space="PSUM")
            nc.tensor.matmul(ppf_ps[:], lt_sb[:], csum[:, J - 1:J], start=True, stop=True)
            ppf = rt.tile([P, 1], f32)
            nc.vector.tensor_copy(ppf[:], ppf_ps[:])
            pos = rt.tile([P, J], f32)
            nc.vector.tensor_scalar_add(pos[:], csum[:], ppf[:, 0:1])
            nc.vector.tensor_sub(pos[:], pos[:], selm[:])
            nc.vector.tensor_scalar_sub(pos[:], pos[:], float(CAP))
            nc.vector.tensor_mul(pos[:], pos[:], selm[:])
            nc.vector.tensor_scalar(pos[:], pos[:], float(CAP), float(CAP),
                                    ALU.add, ALU.min)
            slot_i = rt.tile([P, J], i32)
            nc.vector.tensor_copy(slot_i[:], pos[:])
            tok_i = rt.tile([P, J], i32)
            nc.gpsimd.iota(tok_i[:], pattern=[[1, J]], base=0, channel_multiplier=J)
            pairs = rt.tile([P, J, 2], f32)
            nc.vector.tensor_copy(pairs[:, :, 0], tok_i[:])
            nc.vector.tensor_copy(pairs[:, :, 1], ge[:])
            for j in range(J):
                nc.gpsimd.indirect_dma_start(
                    out=table[:, :],
                    out_offset=bass.IndirectOffsetOnAxis(ap=slot_i[:, j:j + 1], axis=0),
                    in_=pairs[:, j, :], in_offset=None)
            tbl = rt.tile([P, NG, 2], f32)
            nc.sync.dma_start(out=tbl[:, :, :],
                              in_=table[:CAP].rearrange("(g p) c -> p g c", p=P))
            nc.vector.tensor_copy(ids_i[:], tbl[:, :, 0])
            nc.vector.tensor_copy(gslot[:], tbl[:, :, 1])

        px2.__exit__(None, None, None)

        # ======== Phase E: expert FFN (bf16) ========
        with ExitStack() as ph:
            fw = ph.enter_context(tc.tile_pool(name="fw", bufs=1))
            ffp = ph.enter_context(tc.tile_pool(name="ffp", bufs=2))
            fh = ph.enter_context(tc.tile_pool(name="fh", bufs=1))
            psF = ph.enter_context(tc.tile_pool(name="psF", bufs=2, space="PSUM"))

            fc1w_sb = fw.tile([P, DC, F], bf16)
            nc.sync.dma_start(out=fc1w_sb[:, :, :],
                              in_=fc1wt.rearrange("(c p) f -> p c f", p=P))
            fc2w_sb = fw.tile([P, FJ, D], bf16)
            nc.sync.dma_start(out=fc2w_sb[:, :, :],
                              in_=fc2wt.rearrange("(c p) f -> p c f", p=P))
            fc1b_sb = fw.tile([P, FJ], f32)
            nc.sync.dma_start(out=fc1b_sb[:], in_=fc1b_pj[:, :])
            fc2b_sb = fw.tile([P, D], f32)
            nc.sync.dma_start(out=fc2b_sb[:], in_=fc2b[:, :])

            SCN = CAP // (SCH * P)  # 5 chunks of 256 slots
            for sc in range(SCN):
                sraw = ffp.tile([P, SCH, D], bf16, tag="sraw")
                for ss in range(SCH):
                    g = sc * SCH + ss
                    nc.gpsimd.indirect_dma_start(
                        out=sraw[:, ss, :], out_offset=None,
                        in_=nx2_full[:, :],
                        in_offset=bass.IndirectOffsetOnAxis(ap=ids_i[:, g:g + 1], axis=0),
                        bounds_check=NTOK - 1, oob_is_err=False)
                sT = ffp.tile([P, DC, SCH * P], bf16, tag="sT")
                for ss in range(SCH):
                    for dc in range(DC):
                        tp = psF.tile([P, P], bf16, tag="tposeF", space="PSUM")
                        nc.tensor.transpose(tp[:], sraw[:, ss, dc * P:(dc + 1) * P],
                                            identb[:, :])
                        nc.scalar.activation(sT[:, dc, ss * P:(ss + 1) * P], tp[:],
                                             AF.Copy)
                hT = fh.tile([P, FJ, SCH * P], bf16, tag="hT")
                for fj in range(FJ):
                    ps1 = psF.tile([P, SCH * P], f32, tag="ps1", space="PSUM")
                    for dc in range(DC):
                        nc.tensor.matmul(ps1[:], fc1w_sb[:, dc, fj * P:(fj + 1) * P],
                                         sT[:, dc, :], start=(dc == 0),
                                         stop=(dc == DC - 1))
                    nc.scalar.activation(hT[:, fj, :], ps1[:], AF.Gelu,
                                         bias=fc1b_sb[:, fj:fj + 1])
                ysb = ffp.tile([P, SCH, D], bf16, tag="ysb")
                for ss in range(SCH):
                    for dj in range(2):
                        ps2 = psF.tile([P, TOK], f32, tag="ps2", space="PSUM")
                        for fj in range(FJ):
                            nc.tensor.matmul(ps2[:], hT[:, fj, ss * P:(ss + 1) * P],
                                             fc2w_sb[:, fj, dj * TOK:(dj + 1) * TOK],
                                             start=(fj == 0), stop=(fj == FJ - 1))
                        tmp = ffp.tile([P, TOK], f32, tag="ytmp")
                        nc.vector.tensor_add(tmp[:], ps2[:],
                                             fc2b_sb[:, dj * TOK:(dj + 1) * TOK])
                        nc.vector.tensor_scalar_mul(
                            ysb[:, ss, dj * TOK:(dj + 1) * TOK], tmp[:],
                            gslot[:, sc * SCH + ss:sc * SCH + ss + 1])
                for ss in range(SCH):
                    g = sc * SCH + ss
                    nc.gpsimd.indirect_dma_start(
                        out=y_full[:, :],
                        out_offset=bass.IndirectOffsetOnAxis(ap=ids_i[:, g:g + 1], axis=0),
                        in_=ysb[:, ss, :], in_offset=None)

        # ======== ReduceScatter + residual ========
        nc.gpsimd.collective_compute("ReduceScatter", ALU.add, replica_groups=RG8,
                                     ins=[y_full[:NTOK, :]], outs=[rs_out[:, :]])
        with ExitStack() as ph:
            fin = ph.enter_context(tc.tile_pool(name="fin", bufs=1))
            x2r = fin.tile([P, TT, D], f32)
            nc.sync.dma_start(out=x2r[:, :, :],
                              in_=x2_dram.rearrange("(t p) d -> p t d", p=P))
            rsr = fin.tile([P, TT, D], bf16)
            nc.sync.dma_start(out=rsr[:, :, :],
                              in_=rs_out.rearrange("(t p) d -> p t d", p=P))
            ofin = fin.tile([P, TT, D], f32)
            nc.vector.tensor_add(ofin[:, :, :], x2r[:, :, :], rsr[:, :, :])
            nc.sync.dma_start(out=out.rearrange("(t p) d -> p t d", p=P),
                              in_=ofin[:, :, :])
        if dbg:
            nc.gpsimd.dma_start(out=d_x2[:, :], in_=x2_dram[:, :])
            nc.gpsimd.dma_start(out=d_nx2[:, :], in_=nx2_full[:, :])
            nc.gpsimd.dma_start(out=d_g[:, :], in_=g_full[:, :])
            nc.gpsimd.dma_start(out=d_tbl[:, :], in_=table[:, :])
            nc.gpsimd.dma_start(out=d_rs[:, :], in_=rs_out[:, :])
            nc.gpsimd.dma_start(out=d_kv[:, :], in_=kv_send[:, :])

    nc.finalize()
    return nc


_NC_CACHE = None


def _get_nc():
    global _NC_CACHE
    if _NC_CACHE is None:
        _NC_CACHE = build()
    return _NC_CACHE


def kernel(x, ln1_w, ln1_b, ln2_w, ln2_b, Wqkv, bqkv, Wo, bo,
           gate_W, fc1_w, fc1_b, fc2_w, fc2_b):
    x = np.asarray(x, np.float32)
    Wqkv = np.asarray(Wqkv, np.float32)
    bqkv = np.asarray(bqkv, np.float32)
    fc1_w = np.asarray(fc1_w, np.float32)
    fc2_w = np.asarray(fc2_w, np.float32)
    rep = lambda v: np.ascontiguousarray(
        np.broadcast_to(np.asarray(v, np.float32)[None, :], (P, len(v))))

    common = {
        "ln1w": rep(ln1_w), "ln1b": rep(ln1_b),
        "ln2w": rep(ln2_w), "ln2b": rep(ln2_b),
        "wqkv": Wqkv,
        "bq_pj": np.ascontiguousarray(bqkv[:D].reshape(DC, P).T),
        "bkv": rep(bqkv[D:]),
        "wo": np.asarray(Wo, np.float32), "bo": rep(bo),
        "gatew": np.asarray(gate_W, np.float32),
        "ltri": np.triu(np.ones((P, P), np.float32), 1),
    }
    in_maps = []
    for c in range(NC):
        b, h = divmod(c, 2)
        qg = 512 * h + np.arange(TOK)
        kg = np.arange(S)
        mask = np.where(kg[:, None] <= qg[None, :], 0.0, NEG).astype(np.float32)
        kvrow = (1024 * b + np.arange(S)).astype(np.int32).reshape(DC, P).T
        onehot = np.zeros((E,), np.float32)
        onehot[c] = 1.0
        m = dict(common)
        m.update({
            "xown": np.ascontiguousarray(x[b, 512 * h:512 * h + TOK, :]),
            "maskt": mask.astype(ml_dtypes.bfloat16),
            "kvsel": np.ascontiguousarray(kvrow),
            "sel1": np.ascontiguousarray(np.broadcast_to(onehot[None, :], (P, E))),
            "fc1wt": np.ascontiguousarray(fc1_w[c].T).astype(ml_dtypes.bfloat16),
            "fc1b_pj": np.ascontiguousarray(
                np.asarray(fc1_b, np.float32)[c].reshape(FJ, P).T),
            "fc2wt": np.ascontiguousarray(fc2_w[c].T).astype(ml_dtypes.bfloat16),
            "fc2b": rep(np.asarray(fc2_b, np.float32)[c]),
        })
        in_maps.append(m)

    res = run_bass_kernel_spmd(_get_nc(), in_maps, core_ids=list(range(NC)))
    out_flat = np.concatenate([res.results[c]["out"] for c in range(NC)], axis=0)
    return out_flat.reshape(B, S, D).astype(np.float32)
